# revision 1
# baseline (speedup 1.0000x reference)
"""DynamicConvolution TRN2 Bass kernel.

Problem (per reference):
  x: (32, 128, 64, 64) f32
  attention: pooled = mean(x, HW) -> MLP (relu) -> prompt dot -> softmax over K=8
  agg_w[b] = sum_k alpha[b,k] * kernels_weights[k]  (K=8 banks of (128,128,3,3))
  out[b] = conv2d(x[b], agg_w[b], pad=1) + agg_b[b]   -> (32, 128, 64, 64)

Strategy (single-shot ~95us/core, ~81us marginal in TimelineSim; rel err ~4e-3):
  - Data-parallel over batch: 8 cores x 4 samples, identical SPMD program.
  - x and the kernel bank ship as bf16 (halves HBM traffic; bf16 matmuls run
    1 col/cycle like fp32r).  Conv = 9 shifted matmuls per 8-row chunk
    accumulating in fp32 PSUM; per-chunk bias-evict on ACT; per-chunk store
    on the GpSimd DMA queue (keeps the SP queue pure loads so back-to-back
    invocations overlap).
  - The Tile scheduler enqueues per-engine by readiness, so a prologue
    computes all four samples' attention MLPs (alphas broadcast/transposed
    on-chip via tiny matmuls) before the first aggregated weight finishes —
    otherwise later MLP matmuls queue behind entire conv streams.
  - Per-sample bank aggregation on DVE, all-bf16: tensor_scalar muls (4x
    mode) + tensor_tensor adds (2x mode), streamed against the per-bank kw
    DMA arrivals; a zero-multiply scalar_tensor_tensor gate serializes the
    per-sample chains so sample 0's chain (which gates conv0) owns the DVE.
  - Zero padding rows are memset on-chip (DMA ships only the 64 payload
    rows); x0 arrives in quarters + x1-3 in halves so pooled sums (DVE
    quarters for sample 0, ACT accumulate for 1-3) stream with the DMAs.
  - Sparse warmup matmuls gated on successive DMA arrivals hold the PE
    clock-gate open through the prologue so the conv stream starts warm.
  - build(reps=N) replicates the body inside one NEFF for RPC-free timing.
"""
import sys

sys.path.insert(0, "/opt/trn_rl_repo")

import numpy as np
import ml_dtypes

import concourse.bacc as bacc
import concourse.mybir as mybir
import concourse.tile as tile
from concourse.bass_utils import run_bass_kernel_spmd

# problem dims
B, C, H, W = 32, 128, 64, 64
K, KS = 8, 3
HID = 512
NCORES = 8
BL = B // NCORES          # local batch = 4
HP, WP = H + 2, W + 2     # 66x66 padded
NPIX = HP * WP            # 4356
RCHUNK = 8                # output rows per PSUM chunk
NCHUNK = H // RCHUNK      # 8
QROWS = (16, 16, 16, 16)  # x0 DMA quarter row-splits (rows 1-64)
F32 = mybir.dt.float32
BF16 = mybir.dt.bfloat16
AX = mybir.AxisListType
OP = mybir.AluOpType
AF = mybir.ActivationFunctionType


def build(timing_chain: bool = False, probe_skip=(), reps: int = 1):
    """reps>1 replicates the whole body inside one NEFF (for timing: the
    marginal wall-clock between reps=N and reps=1 executables is the true
    per-iteration device time, free of per-call RPC overhead)."""
    nc = bacc.Bacc("TRN2", target_bir_lowering=False, debug=False)

    if timing_chain:
        nc.dram_tensor("chain", [BL, C, H * W], F32, kind="ExternalInput")
    xp = nc.dram_tensor("xp", [BL, C, H * WP], BF16, kind="ExternalInput")
    w1t = nc.dram_tensor("w1t", [C, HID], F32, kind="ExternalInput")
    b1c = nc.dram_tensor("b1c", [C, 4], F32, kind="ExternalInput")
    w2t = nc.dram_tensor("w2t", [C, 4, HID], F32, kind="ExternalInput")
    b2c = nc.dram_tensor("b2c", [C, 4], F32, kind="ExternalInput")
    pt = nc.dram_tensor("pt", [C, 4, K], F32, kind="ExternalInput")
    kb = nc.dram_tensor("kb", [K, C], F32, kind="ExternalInput")
    kw = nc.dram_tensor("kw", [C, K, KS * KS, C], BF16, kind="ExternalInput")
    out = nc.dram_tensor("out", [BL, C, H * W], F32, kind="ExternalOutput")

    taps = [(ti, tj) for ti in range(KS) for tj in range(KS)]

    with tile.TileContext(nc) as tc:
        with (
            tc.tile_pool(name="singles", bufs=1) as singles,
            tc.tile_pool(name="xpool", bufs=2 * BL) as xpool,
            tc.tile_pool(name="opool", bufs=4) as opool,
            tc.tile_pool(name="aggpool", bufs=2) as aggpool,
            tc.tile_pool(name="accpool", bufs=1) as accpool,
            tc.tile_pool(name="kwpool", bufs=2) as kwpool,
            tc.tile_pool(name="scr", bufs=2) as scr,
            tc.tile_pool(name="mlpp", bufs=2, space="PSUM") as mlpp,
            tc.tile_pool(name="convp", bufs=4, space="PSUM") as convp,
            tc.tile_pool(name="warmp", bufs=1, space="PSUM") as warmp,
        ):
            for _rep in range(reps):
                # ---- DMAs in priority order (SP queue = issue order) ----
                x_sb = []
                for _ in range(BL):
                    xt = xpool.tile([C, HP, WP], BF16, tag="x")
                    x_sb.append(xt)
                # zero-pad rows via on-chip memset; DMA only rows 1-64
                for s_ in range(BL):
                    nc.vector.memset(x_sb[s_][:, 0:1, :], 0.0)
                    nc.vector.memset(x_sb[s_][:, H + 1 :, :], 0.0)
                # sample 0 in four quarters so its pooled reduce overlaps the DMA
                r0 = 1
                for q, qr in enumerate(QROWS):
                    nc.sync.dma_start(
                        out=x_sb[0][:, r0 : r0 + qr, :],
                        in_=xp.ap()[0][:, (r0 - 1) * WP : (r0 - 1 + qr) * WP]
                        .rearrange("p (a b) -> p a b", a=qr),
                    )
                    r0 += qr
                w1t_sb = singles.tile([C, HID], F32)
                nc.sync.dma_start(out=w1t_sb, in_=w1t.ap())
                b1_sb = singles.tile([C, 4], F32)
                nc.sync.dma_start(out=b1_sb, in_=b1c.ap())
                w2t_sb = singles.tile([C, 4, HID], F32)
                nc.sync.dma_start(out=w2t_sb, in_=w2t.ap())
                b2_sb = singles.tile([C, 4], F32)
                nc.sync.dma_start(out=b2_sb, in_=b2c.ap())
                pt_sb = singles.tile([C, 4, K], F32)
                nc.sync.dma_start(out=pt_sb, in_=pt.ap())
                kb_sb = singles.tile([K, C], F32)
                nc.sync.dma_start(out=kb_sb, in_=kb.ap())
                def dma_x_quarters(s):
                    # quarters so the ACT pooled-accums stream with the DMA
                    r = 1
                    for qr in QROWS:
                        nc.sync.dma_start(
                            out=x_sb[s][:, r : r + qr, :],
                            in_=xp.ap()[s][:, (r - 1) * WP : (r - 1 + qr) * WP]
                            .rearrange("p (a b) -> p a b", a=qr),
                        )
                        r += qr

                # kw banks 0-5 land early (the sample-0 agg chain drains while
                # x2/x3 stream); banks 6-7 last so aggw0 still completes after
                # every sample's MLP matmuls are ready (scheduler readiness
                # constraint) but with only a ~1.7us post-DMA tail
                kw_sb = kwpool.tile([C, K, KS * KS, C], BF16, tag="kw")
                dma_x_quarters(1)
                for k in range(K - 1):
                    nc.sync.dma_start(out=kw_sb[:, k], in_=kw.ap()[:, k])
                dma_x_quarters(2)
                dma_x_quarters(3)
                nc.sync.dma_start(out=kw_sb[:, K - 1], in_=kw.ap()[:, K - 1])

                # ---- consts / persistent tiles ----
                ones128 = singles.tile([1, 128], F32)
                nc.gpsimd.memset(ones128, 1.0)
                one1 = singles.tile([1, 1], F32)
                nc.gpsimd.memset(one1, 1.0)
                pooled = singles.tile([C, BL], F32)
                junk = singles.tile([C, NPIX], BF16)
                ph = singles.tile([C, 20], F32)
                h_sb = singles.tile([C, 4, BL], F32)
                s_sb = singles.tile([C, 4, BL], F32)
                aggb_sb = singles.tile([C, BL], F32)

                def pooled_s(s):
                    # mean folded into w1t scale host-side; just sums here
                    if s == 0:
                        # quarters on DVE, streaming with the quarter DMAs
                        r0 = 1
                        for q, qr in enumerate(QROWS):
                            nc.vector.tensor_reduce(
                                ph[:, q : q + 1], x_sb[0][:, r0 : r0 + qr, :],
                                axis=AX.XY, op=OP.add,
                            )
                            r0 += qr
                        nc.vector.tensor_reduce(
                            pooled[:, 0:1], ph[:, 0:4], axis=AX.X, op=OP.add
                        )
                    else:
                        # ACT accumulate per quarter, streaming with the DMAs
                        c0 = 4 * s
                        r = 1
                        for q, qr in enumerate(QROWS):
                            nc.scalar.activation(
                                junk[:, (r - 1) * WP : (r - 1 + qr) * WP],
                                x_sb[s][:, r : r + qr, :]
                                .rearrange("p a b -> p (a b)"),
                                AF.Copy, accum_out=ph[:, c0 + q : c0 + q + 1],
                            )
                            r += qr
                        nc.vector.tensor_reduce(
                            pooled[:, s : s + 1], ph[:, c0 : c0 + 4],
                            axis=AX.X, op=OP.add,
                        )

                def mlp_make(s, state):
                    """Per-sample attention MLP as 3 burst stages (so stages can
                    interleave into the previous sample's conv chunk stream
                    without head-of-line blocking any engine FIFO)."""

                    def h_stage():
                        M1 = mlpp.tile([C, 32], F32, tag="mlp")
                        state["M1"] = M1
                        for c in range(4):
                            nc.tensor.matmul(
                                M1[:, c : c + 1], w1t_sb[:, 128 * c : 128 * (c + 1)],
                                pooled[:, s : s + 1], start=True, stop=True,
                            )
                        for c in range(4):
                            nc.vector.tensor_scalar(
                                h_sb[:, c, s : s + 1], M1[:, c : c + 1],
                                b1_sb[:, c : c + 1], 0.0, op0=OP.add, op1=OP.max,
                            )

                    def s_stage():
                        M1 = state["M1"]
                        for c2 in range(4):
                            for c in range(4):
                                nc.tensor.matmul(
                                    M1[:, 4 + c2 : 5 + c2],
                                    w2t_sb[:, c, 128 * c2 : 128 * (c2 + 1)],
                                    h_sb[:, c, s : s + 1],
                                    start=(c == 0), stop=(c == 3),
                                )
                        for c2 in range(4):
                            nc.vector.tensor_scalar_add(
                                s_sb[:, c2, s : s + 1], M1[:, 4 + c2 : 5 + c2],
                                b2_sb[:, c2 : c2 + 1],
                            )
                        for c2 in range(4):
                            nc.tensor.matmul(
                                M1[0:1, 16:24], s_sb[:, c2, s : s + 1],
                                pt_sb[:, c2, :],
                                start=(c2 == 0), stop=(c2 == 3),
                            )

                    def sm_stage():
                        M1 = state["M1"]
                        negmx = scr.tile([1, 1], F32, tag="negmx")
                        nc.vector.tensor_reduce(
                            negmx, M1[0:1, 16:24], axis=AX.X, op=OP.max, negate=True
                        )
                        ex = scr.tile([1, K], F32, tag="ex")
                        nc.scalar.activation(ex, M1[0:1, 16:24], AF.Exp, bias=negmx)
                        sm = scr.tile([1, 1], F32, tag="sm")
                        nc.vector.tensor_reduce(sm, ex, axis=AX.X, op=OP.add)
                        rsm = scr.tile([1, 1], F32, tag="rsm")
                        nc.vector.reciprocal(rsm, sm)
                        alphas = scr.tile([1, K], F32, tag="alphas")
                        nc.vector.tensor_scalar_mul(alphas, ex, rsm)
                        # broadcast alphas to all 128 partitions (PE)
                        nc.tensor.matmul(
                            M1[:, 8:16], ones128, alphas, start=True, stop=True
                        )
                        a_bc = scr.tile([C, K], F32, tag="abc")
                        nc.scalar.copy(a_bc, M1[:, 8:16])
                        # k onto partitions (PE transpose via matmul with ones)
                        nc.tensor.matmul(
                            M1[0:8, 24:25], alphas, one1, start=True, stop=True
                        )
                        ak8 = scr.tile([8, 1], F32, tag="ak8")
                        nc.scalar.copy(ak8, M1[0:8, 24:25])
                        # aggregated bias: kb.T @ alpha
                        nc.tensor.matmul(
                            M1[:, 25:26], kb_sb, ak8, start=True, stop=True
                        )
                        nc.scalar.copy(aggb_sb[:, s : s + 1], M1[:, 25:26])
                        state["abc"] = a_bc

                    return h_stage, s_stage, sm_stage

                def agg_s(s, a_bc, prev_aggw):
                    """Weighted sum of the 8 kernel banks on DVE, all-bf16: muls
                    are tensor_scalar 4x mode (360ns), adds tensor_tensor 2x mode
                    (660ns).  Banks 0-5 chain while their DMAs stream; banks 6-7
                    pair off-chain so the post-DMA critical path is short.  For
                    s>0 the alpha vector is copied through a zero-multiply of the
                    previous aggw, serializing the per-sample chains so sample
                    0's chain (which gates conv0) owns the DVE."""
                    if prev_aggw is not None:
                        # all-f32 zero-multiply gate: bitcast pairs of prev_aggw's
                        # bf16s to f32 (finite by construction) purely to carry a
                        # data dependency on the previous sample's aggregation
                        gated = scr.tile([C, K], F32, tag="abcg")
                        nc.vector.scalar_tensor_tensor(
                            gated, prev_aggw[:, 0, 0 : 2 * K].bitcast(F32), 0.0,
                            a_bc, op0=OP.mult, op1=OP.add,
                        )
                        a_bc = gated
                    acc = None
                    tmps = []
                    for k in range(K - 2):
                        tk = aggpool.tile([C, KS * KS, C], BF16, tag=f"tmp{k % 2}")
                        nc.vector.tensor_scalar_mul(tk, kw_sb[:, k], a_bc[:, k : k + 1])
                        tmps.append(tk)
                        if k == 1:
                            acc = accpool.tile([C, KS * KS, C], BF16, tag="accA")
                            nc.vector.tensor_tensor(acc, tmps[0], tmps[1], op=OP.add)
                        elif k > 1:
                            nxt = accpool.tile(
                                [C, KS * KS, C], BF16,
                                tag="accA" if k % 2 else "accB",
                            )
                            nc.vector.tensor_tensor(nxt, acc, tk, op=OP.add)
                            acc = nxt
                    t6 = aggpool.tile([C, KS * KS, C], BF16, tag="tmp0")
                    nc.vector.tensor_scalar_mul(t6, kw_sb[:, K - 2], a_bc[:, K - 2 : K - 1])
                    t7 = aggpool.tile([C, KS * KS, C], BF16, tag="tmp1")
                    nc.vector.tensor_scalar_mul(t7, kw_sb[:, K - 1], a_bc[:, K - 1 : K])
                    t67 = accpool.tile([C, KS * KS, C], BF16, tag="accB")
                    nc.vector.tensor_tensor(t67, t6, t7, op=OP.add)
                    aggw = aggpool.tile([C, KS * KS, C], BF16, tag="aggw")
                    nc.vector.tensor_tensor(aggw, acc, t67, op=OP.add)
                    return aggw

                def conv_s(s, aggw, hooks=None):
                    for chunk in range(NCHUNK):
                        h0 = chunk * RCHUNK
                        ps_c = convp.tile([C, RCHUNK, W], F32, tag="ps_c")
                        for t, (ti, tj) in enumerate(taps):
                            nc.tensor.matmul(
                                ps_c, aggw[:, t, :],
                                x_sb[s][:, h0 + ti : h0 + ti + RCHUNK, tj : tj + W],
                                start=(t == 0), stop=(t == KS * KS - 1),
                            )
                        oc = opool.tile([C, RCHUNK, W], F32, tag="oc")
                        last = s == BL - 1 and chunk == NCHUNK - 1
                        if last:
                            # final chunk: halve the evict+store pipeline so
                            # the kernel tail is one 4-row store, not 8
                            for hh in (0, RCHUNK // 2):
                                nc.scalar.activation(
                                    oc[:, hh : hh + RCHUNK // 2, :],
                                    ps_c[:, hh : hh + RCHUNK // 2, :],
                                    AF.Identity, bias=aggb_sb[:, s : s + 1],
                                )
                                nc.sync.dma_start(
                                    out=out.ap()[s][
                                        :, (h0 + hh) * W : (h0 + hh + RCHUNK // 2) * W
                                    ],
                                    in_=oc[:, hh : hh + RCHUNK // 2, :]
                                    .rearrange("p a b -> p (a b)"),
                                )
                        else:
                            nc.scalar.activation(
                                oc, ps_c, AF.Identity, bias=aggb_sb[:, s : s + 1]
                            )
                            nc.sync.dma_start(
                                out=out.ap()[s][:, h0 * W : (h0 + RCHUNK) * W],
                                in_=oc.rearrange("p a b -> p (a b)"),
                            )
                        if hooks and chunk in hooks:
                            hooks[chunk]()

                # ---- interleaved schedule (engine FIFOs follow program order) ----
                # PE warmup: throwaway matmuls keep the PE clock-gate hot
                # through the prologue so the conv stream starts at full rate.
                wlhs = x_sb[0].rearrange("p a b -> p (a b)")[:, 128:256]
                wps = warmp.tile([C, 8, W], F32, tag="warm")
                # sparse: one ~213ns dummy per arriving transfer keeps the PE
                # clock-gate window busy (gaps stay under ~3.4us) without
                # stealing meaningful PE time from the real work
                warm_rhs = [
                    x_sb[0][:, 25:33, 1 : 1 + W],
                    x_sb[0][:, 56:64, 1 : 1 + W],
                    x_sb[1][:, 56:64, 1 : 1 + W],
                    x_sb[2][:, 56:64, 1 : 1 + W],
                    x_sb[3][:, 56:64, 1 : 1 + W],
                    x_sb[3][:, 25:33, 1 : 1 + W],
                ] + [kw_sb[:, k, 0:4, :] for k in (2, 5)]
                for rhs_ in warm_rhs:
                    nc.tensor.matmul(wps, wlhs, rhs_, start=True, stop=True)
                nc.tensor.matmul(
                    wps, w1t_sb[:, 0:128], w1t_sb[:, 0:512],
                    start=True, stop=True,
                )
                nc.tensor.matmul(
                    wps, w2t_sb[:, 0, 0:128],
                    w2t_sb[:, 3, 0:512], start=True, stop=True,
                )

                # Prologue: all four alpha pipelines complete before aggw0
                # does, so the scheduler's readiness order puts every MLP op
                # ahead of the conv streams on each engine.
                sts = [{}, {}, {}, {}]
                aggws = []
                for s_ in range(BL):
                    pooled_s(s_)
                    hs, ss, sms = mlp_make(s_, sts[s_])
                    hs(); ss(); sms()
                    aggws.append(
                        agg_s(s_, sts[s_]["abc"], aggws[-1] if aggws else None)
                    )
                for s_ in range(BL):
                    conv_s(s_, aggws[s_])

    nc.compile()
    return nc


_NC = None


def _get_nc():
    global _NC
    if _NC is None:
        _NC = build()
    return _NC


def prep_inputs(x, prompt_param, w1, b1, w2, b2, kernels_weights, kernels_bias):
    """Host-side layout transforms -> per-core in_maps."""
    x = np.asarray(x, np.float32)
    prompt = np.asarray(prompt_param, np.float32)[0]          # (K, HID)
    w1 = np.asarray(w1, np.float32)
    b1 = np.asarray(b1, np.float32)
    w2 = np.asarray(w2, np.float32)
    b2 = np.asarray(b2, np.float32)
    kwt = np.asarray(kernels_weights, np.float32)             # (K, C, C, 3, 3)
    kbt = np.asarray(kernels_bias, np.float32)                # (K, C)

    w1t = np.ascontiguousarray(w1.T) * np.float32(1.0 / (H * W))  # (C, HID)
    b1c = np.ascontiguousarray(b1.reshape(4, C).T)            # (C, 4)
    w2t = np.ascontiguousarray(w2.T.reshape(4, C, HID).transpose(1, 0, 2))
    b2c = np.ascontiguousarray(b2.reshape(4, C).T)
    pt = np.ascontiguousarray(prompt.T.reshape(4, C, K).transpose(1, 0, 2))
    kwb = np.ascontiguousarray(
        kwt.transpose(2, 0, 3, 4, 1).reshape(C, K, KS * KS, C)
    ).astype(ml_dtypes.bfloat16)
    kb = np.ascontiguousarray(kbt)

    in_maps = []
    for c in range(NCORES):
        xs = x[c * BL : (c + 1) * BL]                          # (4, C, H, W)
        xpad = np.zeros((BL, C, H, WP), ml_dtypes.bfloat16)
        xpad[:, :, :, 1 : W + 1] = xs.astype(ml_dtypes.bfloat16)
        xpad = xpad.reshape(BL, C, H * WP)
        in_maps.append(
            {
                "xp": xpad, "w1t": w1t, "b1c": b1c, "w2t": w2t, "b2c": b2c,
                "pt": pt, "kb": kb, "kw": kwb,
            }
        )
    return in_maps


def kernel(**inputs) -> np.ndarray:
    nc = _get_nc()
    in_maps = prep_inputs(**inputs)
    res = run_bass_kernel_spmd(nc, in_maps, core_ids=list(range(NCORES)))
    outs = [res.results[c]["out"].reshape(BL, C, H, W) for c in range(NCORES)]
    return np.concatenate(outs, axis=0)


if __name__ == "__main__":
    import reference

    inputs = {k: np.asarray(v) for k, v in reference.setup_inputs().items()}
    expected = np.asarray(reference.reference(**inputs))
    actual = kernel(**inputs)
    scale = np.abs(expected).max()
    err = np.abs(actual - expected).max()
    print(f"absmax={err:.3e} scale={scale:.3f} rel={err / scale:.3e}")

